# revision 1
# baseline (speedup 1.0000x reference)
"""Multi-head causal self-attention for TRN2, 8 NeuronCores.

Sharding: core i handles (batch b = i//2, head-group g = i%2); each head-group
is 8 of the 16 heads.  Per core everything is computed in "transposed" space so
no on-device transposes are needed:

  phase 1:  Q^T, K^T [512, T] = W_{q,k}^T @ x^T   (lhsT = W rows, rhs = x^T)
            V [T, 512] = x @ W_v                  (lhsT = x^T chunk, rhs = W_v)
            Q^T stored per-head zero-padded to 128 partitions so attention
            matmuls are full 128x128 shapes (keeps the PE HAM clock-gate warm);
            V staged bf16 as [V_h | 1] per head (+ones tail) for the softmax
            row-sum trick with a full M=128 stationary operand.
  phase 2 (per j-block of 512 query positions):
            per head-pair, per tk-chunk c: S^T(2 heads) = kT_c.T @ qTp into one
            [128,1024] PSUM tile; one ACT exp -> bf16 P^T; causal mask multiply
            on diagonal chunks (DVE); PV accumulate [V_h|1|..].T @ P^T (full
            128x128 bf16) giving O^T rows 0:63 + softmax sums in row 64;
            normalize via approx-reciprocal + K=1 broadcast matmul + DVE mul
            into bf16 Y^T; then the output projection rows for this j-block
            (Y^T.T @ W_proj in bf16) ride behind the ACT-bound attention.
  host sums the two head-group partials per batch and adds b_proj.

QKV/attention-score matmuls in float32r (4x faster than fp32, ~1.5e-4 err);
P/V and the projection in bf16.
"""

import numpy as np
import ml_dtypes
from contextlib import ExitStack

import concourse.bass as bass
import concourse.mybir as mybir
import concourse.tile as tile
from concourse import bacc
from concourse.bass_utils import run_bass_kernel_spmd

B, T, D, H = 4, 2048, 1024, 16
DK = 64            # head dim
HL = 8             # heads per core
DL = HL * DK       # 512 local head dims per core
N_CORES = 8

F32 = mybir.dt.float32
F32R = mybir.dt.float32r
BF16 = mybir.dt.bfloat16
EXP = mybir.ActivationFunctionType.Exp
IDENT = mybir.ActivationFunctionType.Identity

TQ = 512           # tq block size
TKC = 128          # tk chunk size
NQB = T // TQ      # 4
NKC = T // TKC     # 16
NDCH = D // 128    # 8 contraction chunks over D
VSW = HL * 65 + 64  # staged-V width: 8*[V_h|1] + ones tail pad for M=128 lhsT

_CACHE = {}


def _build(causal: bool):
    nc = bacc.Bacc("TRN2", target_bir_lowering=False, debug=False,
                   num_devices=N_CORES)
    xT_d = nc.dram_tensor("xT", [D, T], F32, kind="ExternalInput").ap()
    wqk_d = nc.dram_tensor("wqk", [D, 2 * DL], F32, kind="ExternalInput").ap()
    wv_d = nc.dram_tensor("wv", [D, DL], F32, kind="ExternalInput").ap()
    bqk_d = nc.dram_tensor("bqk", [2 * DL // 128, 128, 1], F32,
                           kind="ExternalInput").ap()
    bv_d = nc.dram_tensor("bv", [1, DL], F32, kind="ExternalInput").ap()
    wp_d = nc.dram_tensor("wproj", [DL, D], F32, kind="ExternalInput").ap()
    masks_d = nc.dram_tensor("masks", [TKC, 4 * TQ], BF16,
                             kind="ExternalInput").ap()
    out_d = nc.dram_tensor("out", [T, D], F32, kind="ExternalOutput").ap()

    with tile.TileContext(nc) as tc, ExitStack() as top:
        persist = top.enter_context(tc.tile_pool(name="persist", bufs=1))

        qTp = [persist.tile([128, T], BF16, tag=f"qTp{h}", name=f"qTp{h}")
               for h in range(HL)]      # per-head, zero-padded other half
        kT = [persist.tile([128, T], BF16, tag=f"kT{i}", name=f"kT{i}")
              for i in range(4)]        # head-pair packed
        vs = [persist.tile([128, VSW], BF16, tag=f"vs{t}", name=f"vs{t}")
              for t in range(NKC)]
        ones_r = persist.tile([1, 128], F32R, tag="ones_r", name="ones_r")
        bqk_sb = [persist.tile([128, 1], F32, tag=f"bqk{m}", name=f"bqk{m}")
                  for m in range(8)]
        for m in range(8):
            nc.gpsimd.dma_start(bqk_sb[m][:], bqk_d[m])
        bv_r = persist.tile([1, DL], F32R, tag="bv_r", name="bv_r")

        # ---------------- phase 1: QKV projections ----------------
        with ExitStack() as ph1:
            wstage = ph1.enter_context(tc.tile_pool(name="wstage", bufs=3))
            wpool = ph1.enter_context(tc.tile_pool(name="wpool", bufs=1))
            xstage = ph1.enter_context(tc.tile_pool(name="xstage", bufs=4))
            xrpool = ph1.enter_context(tc.tile_pool(name="xrpool", bufs=2))
            ps1 = ph1.enter_context(tc.tile_pool(name="ps1", bufs=3, space="PSUM"))

            # small constants + one-time fills
            initp = ph1.enter_context(tc.tile_pool(name="initp", bufs=1))
            ones_f = initp.tile([1, 128], F32, tag="ones_f", name="ones_f")
            nc.vector.memset(ones_f[:], 1.0)
            nc.vector.tensor_copy(ones_r[:], ones_f[:])
            ones8 = initp.tile([128, 64], F32, tag="ones8", name="ones8")
            nc.vector.memset(ones8[:], 1.0)
            bv_f = initp.tile([1, DL], F32, tag="bv_f", name="bv_f")
            nc.gpsimd.dma_start(bv_f[:], bv_d)
            nc.vector.tensor_copy(bv_r[:], bv_f[:])
            zeros = initp.tile([64, TQ], F32, tag="zeros", name="zeros")
            nc.vector.memset(zeros[:], 0.0)
            for h in range(HL):
                pad = slice(64, 128) if h % 2 == 0 else slice(0, 64)
                for jz in range(NQB):
                    nc.vector.tensor_copy(
                        qTp[h][pad, jz * TQ:(jz + 1) * TQ], zeros[:])
            for t in range(NKC):
                for h in range(HL):
                    nc.vector.tensor_copy(
                        vs[t][:, h * 65 + 64:h * 65 + 65], ones8[:, 0:1])
                nc.vector.tensor_copy(vs[t][:, HL * 65:VSW], ones8[:])

            wqk_r, wv_r = [], []
            for d in range(NDCH):
                st = wstage.tile([128, 2 * DL], F32, tag="wqks", name=f"wqks{d}")
                nc.gpsimd.dma_start(st[:], wqk_d[d * 128:(d + 1) * 128, :])
                wr = wpool.tile([128, 2 * DL], BF16, tag=f"wqk{d}", name=f"wqk{d}")
                nc.vector.tensor_copy(wr[:], st[:])
                wqk_r.append(wr)

                stv = wstage.tile([128, DL], F32, tag="wvs", name=f"wvs{d}")
                nc.gpsimd.dma_start(stv[:], wv_d[d * 128:(d + 1) * 128, :])
                wvr = wpool.tile([128, DL], BF16, tag=f"wv{d}", name=f"wv{d}")
                nc.vector.tensor_copy(wvr[:], stv[:])
                wv_r.append(wvr)

            for j in range(NQB):
                jsl = slice(j * TQ, (j + 1) * TQ)
                xr = []
                for d in range(NDCH):
                    st = xstage.tile([128, TQ], F32, tag="xs", name=f"xs{j}_{d}")
                    nc.sync.dma_start(st[:], xT_d[d * 128:(d + 1) * 128, jsl])
                    xrt = xrpool.tile([128, TQ], BF16, tag=f"xr{d}",
                                      name=f"xr{j}_{d}")
                    nc.vector.tensor_copy(xrt[:], st[:])
                    xr.append(xrt)

                for m in range(8):
                    ps = ps1.tile([128, TQ], F32, tag="psqk", name=f"psqk{j}_{m}")
                    for d in range(NDCH):
                        nc.tensor.matmul(
                            ps[:], wqk_r[d][:, m * 128:(m + 1) * 128],
                            xr[d][:], start=(d == 0), stop=(d == NDCH - 1))
                    if m < 4:
                        nc.scalar.activation(
                            qTp[2 * m][0:64, jsl], ps[0:64, :], IDENT,
                            bias=bqk_sb[m][0:64], scale=1.0)
                        nc.scalar.activation(
                            qTp[2 * m + 1][64:128, jsl], ps[64:128, :], IDENT,
                            bias=bqk_sb[m][64:128], scale=1.0)
                    else:
                        nc.scalar.activation(
                            kT[m - 4][:, jsl], ps[:], IDENT,
                            bias=bqk_sb[m][:], scale=1.0)

                for tt in range(4 * j, 4 * j + 4):
                    c = tt % 4
                    ps = ps1.tile([128, DL], F32, tag="psv", name=f"psv{tt}")
                    for d in range(NDCH):
                        nc.tensor.matmul(
                            ps[:], xr[d][:, c * 128:(c + 1) * 128], wv_r[d][:],
                            start=(d == 0), stop=False)
                    nc.tensor.matmul(ps[:], ones_r[:, 0:128], bv_r[:],
                                     start=False, stop=True)
                    for h in range(HL):
                        nc.vector.tensor_copy(vs[tt][:, h * 65:h * 65 + 64],
                                              ps[:, h * 64:(h + 1) * 64])

        # -------- phase 2: attention + projection per j-block --------
        with ExitStack() as ph2:
            maskpool = ph2.enter_context(tc.tile_pool(name="maskpool", bufs=1))
            wpool3 = ph2.enter_context(tc.tile_pool(name="wpool3", bufs=1))
            wstage3 = ph2.enter_context(tc.tile_pool(name="wstage3", bufs=2))
            ypool = ph2.enter_context(tc.tile_pool(name="ypool", bufs=1))
            ps_s = ph2.enter_context(tc.tile_pool(name="ps_s", bufs=2, space="PSUM"))
            ps_o = ph2.enter_context(tc.tile_pool(name="ps_o", bufs=2, space="PSUM"))
            ps_b = ph2.enter_context(tc.tile_pool(name="ps_b", bufs=1, space="PSUM"))
            ps_3 = ph2.enter_context(tc.tile_pool(name="ps_3", bufs=1, space="PSUM"))
            ppool = ph2.enter_context(tc.tile_pool(name="ppool", bufs=6))
            npool = ph2.enter_context(tc.tile_pool(name="npool", bufs=3))
            opool = ph2.enter_context(tc.tile_pool(name="opool", bufs=3))

            maskb = None
            if causal:
                maskb = maskpool.tile([TKC, 4 * TQ], BF16, tag="maskb",
                                      name="maskb")
                nc.gpsimd.dma_start(maskb[:], masks_d)
            yT = [ypool.tile([128, T], BF16, tag=f"yT{i}", name=f"yT{i}")
                  for i in range(4)]
            wp_r = []
            for k in range(4):
                st = wstage3.tile([128, D], F32, tag="wps", name=f"wps{k}")
                nc.gpsimd.dma_start(st[:], wp_d[k * 128:(k + 1) * 128, :])
                wr = wpool3.tile([128, D], BF16, tag=f"wp{k}", name=f"wp{k}")
                nc.vector.tensor_copy(wr[:], st[:])
                wp_r.append(wr)

            def proj_step(t, nb):
                nsl = slice(nb * 512, (nb + 1) * 512)
                ps = ps_3.tile([128, TQ], F32, tag="p3", name=f"ps3_{t}_{nb}")
                for k in range(4):
                    nc.tensor.matmul(
                        ps[:], yT[k][:, t * 128:(t + 1) * 128],
                        wp_r[k][:, nsl], start=(k == 0), stop=(k == 3))
                ot = opool.tile([128, TQ], F32, tag="ot", name=f"ot{t}_{nb}")
                nc.vector.tensor_copy(ot[:], ps[:])
                nc.sync.dma_start(out_d[t * 128:(t + 1) * 128, nsl], ot[:])

            pending = []   # proj steps of block j-1, interleaved into attn(j)
            for j in range(NQB):
                jsl = slice(j * TQ, (j + 1) * TQ)
                cs = list(range(4 * (j + 1))) if causal else list(range(NKC))
                for i in range(4):          # head pair (2i, 2i+1)
                    hA, hB = 2 * i, 2 * i + 1
                    poA = ps_o.tile([128, TQ], F32, tag="po", name=f"poA{j}_{i}")
                    poB = ps_o.tile([128, TQ], F32, tag="po", name=f"poB{j}_{i}")

                    pend = None   # pipeline: PV(c) emitted after QK(c+1)
                    for ci, c in enumerate(cs):
                        csl = slice(c * TKC, (c + 1) * TKC)
                        ss = ps_s.tile([TKC, 2 * TQ], F32, tag="ss",
                                       name=f"ss{j}_{i}_{c}")
                        nc.tensor.matmul(ss[:, 0:TQ], kT[i][:, csl],
                                         qTp[hA][:, jsl], start=True, stop=True)
                        nc.tensor.matmul(ss[:, TQ:2 * TQ], kT[i][:, csl],
                                         qTp[hB][:, jsl], start=True, stop=True)
                        pt = ppool.tile([TKC, 2 * TQ], BF16, tag="pt",
                                        name=f"pt{j}_{i}_{c}")
                        nc.scalar.activation(pt[:], ss[:], EXP, scale=0.125)
                        if causal and c >= 4 * j:
                            s = c - 4 * j
                            msl = slice(s * TQ, (s + 1) * TQ)
                            nc.vector.tensor_mul(pt[:, 0:TQ], pt[:, 0:TQ],
                                                 maskb[:, msl])
                            nc.vector.tensor_mul(pt[:, TQ:2 * TQ],
                                                 pt[:, TQ:2 * TQ], maskb[:, msl])
                        if pend is not None:
                            pc, ppt = pend
                            st = (ci == 1)
                            nc.tensor.matmul(
                                poA[:], vs[pc][:, hA * 65:hA * 65 + 128],
                                ppt[:, 0:TQ], start=st, stop=False)
                            nc.tensor.matmul(
                                poB[:], vs[pc][:, hB * 65:hB * 65 + 128],
                                ppt[:, TQ:2 * TQ], start=st, stop=False)
                        if pending:
                            pending.pop(0)()
                        pend = (c, pt)
                    pc, ppt = pend
                    one = (len(cs) == 1)
                    nc.tensor.matmul(poA[:], vs[pc][:, hA * 65:hA * 65 + 128],
                                     ppt[:, 0:TQ], start=one, stop=True)
                    nc.tensor.matmul(poB[:], vs[pc][:, hB * 65:hB * 65 + 128],
                                     ppt[:, TQ:2 * TQ], start=one, stop=True)

                    # normalize: copy sums+O off PSUM fast, approx-recip in SBUF
                    for h, po in ((hA, poA), (hB, poB)):
                        hp = h % 2
                        sums = npool.tile([1, TQ], F32, tag="sums",
                                          name=f"sm{j}_{h}")
                        nc.vector.tensor_copy(sums[:], po[64:65, :])
                        o_sb = npool.tile([64, TQ], BF16, tag="o_sb",
                                          name=f"ob{j}_{h}")
                        nc.vector.tensor_copy(o_sb[:], po[0:64, :])
                        recip = npool.tile([1, TQ], F32, tag="recip",
                                           name=f"rc{j}_{h}")
                        scr = npool.tile([1, TQ], F32, tag="scr",
                                         name=f"sc{j}_{h}")
                        recip_r = npool.tile([1, TQ], F32R, tag="recip_r",
                                             name=f"rr{j}_{h}")
                        nc.vector.reciprocal_approx_accurate(
                            out=recip[:], in_=sums[:], scratch=scr[:])
                        nc.vector.tensor_copy(recip_r[:], recip[:])
                        pb = ps_b.tile([64, TQ], F32, tag="pb",
                                       name=f"pb{j}_{h}")
                        nc.tensor.matmul(pb[:], ones_r[:, 0:64], recip_r[:],
                                         start=True, stop=True)
                        nc.vector.tensor_mul(
                            yT[i][hp * 64:(hp + 1) * 64, jsl], o_sb[:], pb[:])

                # queue this j-block's projection; emitted inside attn(j+1)
                for t in range(4 * j, 4 * j + 4):
                    for nb in range(2):
                        pending.append(
                            lambda t=t, nb=nb: proj_step(t, nb))
            for fn in pending:   # flush last block's projection
                fn()

    nc.compile()
    return nc


def _get_nc(causal: bool):
    if causal not in _CACHE:
        _CACHE[causal] = _build(causal)
    return _CACHE[causal]


def _host_masks() -> np.ndarray:
    i = np.arange(TKC)[:, None]
    jj = np.arange(TQ)[None, :]
    blocks = [(jj >= i + s * TKC).astype(np.float32) for s in range(4)]
    return np.ascontiguousarray(
        np.concatenate(blocks, axis=1).astype(ml_dtypes.bfloat16))


def _make_in_maps(x, W_qkv, b_qkv, W_proj):
    masks_np = _host_masks()
    in_maps = []
    for core in range(N_CORES):
        b, g = core // 2, core % 2
        qc = slice(g * DL, (g + 1) * DL)
        kc = slice(D + g * DL, D + (g + 1) * DL)
        vc = slice(2 * D + g * DL, 2 * D + (g + 1) * DL)
        in_maps.append({
            "xT": np.ascontiguousarray(x[b].T),
            "wqk": np.ascontiguousarray(
                np.concatenate([W_qkv[:, qc], W_qkv[:, kc]], axis=1)),
            "wv": np.ascontiguousarray(W_qkv[:, vc]),
            "bqk": np.ascontiguousarray(
                np.concatenate([b_qkv[qc], b_qkv[kc]]).reshape(8, 128, 1)),
            "bv": np.ascontiguousarray(b_qkv[vc].reshape(1, DL)),
            "wproj": np.ascontiguousarray(W_proj[g * DL:(g + 1) * DL, :]),
            "masks": masks_np,
        })
    return in_maps


def kernel(x, mask, W_qkv, b_qkv, W_proj, b_proj):
    x = np.asarray(x, dtype=np.float32)
    mask2d = np.asarray(mask, dtype=np.int32).reshape(T, T)
    W_qkv = np.asarray(W_qkv, dtype=np.float32)
    b_qkv = np.asarray(b_qkv, dtype=np.float32)
    W_proj = np.asarray(W_proj, dtype=np.float32)
    b_proj = np.asarray(b_proj, dtype=np.float32)

    if np.array_equal(mask2d, np.tril(np.ones((T, T), dtype=np.int32))):
        causal = True
    elif np.all(mask2d == 1):
        causal = False
    else:
        raise NotImplementedError("only causal (tril) or all-ones masks")

    nc = _get_nc(causal)
    in_maps = _make_in_maps(x, W_qkv, b_qkv, W_proj)
    res = run_bass_kernel_spmd(nc, in_maps, core_ids=list(range(N_CORES)))
    out = np.empty((B, T, D), dtype=np.float32)
    for b in range(B):
        out[b] = (res.results[2 * b]["out"] + res.results[2 * b + 1]["out"]
                  + b_proj[None, :])
    return out



# revision 20
# speedup vs baseline: 1.0451x; 1.0451x over previous
"""Multi-head causal self-attention for TRN2, 8 NeuronCores.

Sharding: core i handles (batch b = i//2, head-group g = i%2); each head-group
is 8 of the 16 heads.  Everything on-device is computed in "transposed" space
(features on partitions, positions on the free dim) so no transposes are
needed.  Single fused pipeline per query block j (no separate phases):

  QKV(j+1) and proj(j-1) matmuls are interleaved into attention(j) so the PE
  never idles while the scalar engine chews through the softmax exps.

  scores:  S^T(h) = kT.T @ qT per head, K=64.  The two heads of a pair are
           packed into partitions 0:64 / 64:128 of one kT/qT tile and issued
           as two row-tiled matmuls (tile_position (0,0)/(64,0)) that run
           CONCURRENTLY in the PE array -> ~2x score throughput.
  softmax: one exp per (chunk, pair) over [128, <=1024] PSUM; on diagonal
           chunks the column range is restricted to the causally live region
           and only the 128-wide triangle gets a mask multiply.
  PV:      stationary operand is [V_h (64 cols) | ones (64 cols)] via a
           strided AP, so PSUM rows 0:63 accumulate O^T and rows 64:127 all
           accumulate the softmax denominator -- already broadcast.  The
           normalize is then just reciprocal_approx_fast + tensor_tensor
           multiply on DVE (no PE broadcast matmul, no single-lane ops).
  proj:    y^T.T @ W_proj in bf16, partial outputs summed on host.

All inputs staged as bf16 on the host (halves DMA, removes all on-device
weight/x casts).  b_qkv handled via DVE per-partition bias add (Q/K) and a
K=1 ones matmul (V).  b_proj added on host.
"""

import numpy as np
import ml_dtypes
from collections import deque
from contextlib import ExitStack

import concourse.bass as bass
import concourse.mybir as mybir
import concourse.tile as tile
from concourse import bacc
from concourse.bass_utils import run_bass_kernel_spmd

B, T, D, H = 4, 2048, 1024, 16
DK = 64            # head dim
HL = 8             # heads per core
DL = HL * DK       # 512 local head dims per core
N_CORES = 8

F32 = mybir.dt.float32
F32R = mybir.dt.float32r
BF16 = mybir.dt.bfloat16
EXP = mybir.ActivationFunctionType.Exp

TQ = 512           # query block size
TKC = 128          # key chunk size
NQB = T // TQ      # 4
NKC = T // TKC     # 16
NDCH = D // 128    # 8 contraction chunks over D
_CACHE = {}


def _build(causal: bool):
    nc = bacc.Bacc("TRN2", target_bir_lowering=False, debug=False,
                   num_devices=N_CORES)
    xT_d = nc.dram_tensor("xT", [D, T], BF16, kind="ExternalInput").ap()
    wqk_d = nc.dram_tensor("wqk", [D, 2 * DL], BF16, kind="ExternalInput").ap()
    wv_d = nc.dram_tensor("wv", [D, DL], BF16, kind="ExternalInput").ap()
    bqk_d = nc.dram_tensor("bqk", [128, 8], F32, kind="ExternalInput").ap()
    bv_d = nc.dram_tensor("bv", [1, DL], F32, kind="ExternalInput").ap()
    wp_d = nc.dram_tensor("wproj", [DL, D], BF16, kind="ExternalInput").ap()
    maskt_d = nc.dram_tensor("maskt", [TKC, 2 * TKC], BF16,
                             kind="ExternalInput").ap()
    out_d = nc.dram_tensor("out", [T, D], F32, kind="ExternalOutput").ap()
    sink_d = nc.dram_tensor("sink", [128, 32], F32, kind="ExternalOutput").ap()

    with tile.TileContext(nc) as tc, ExitStack() as top:
        persist = top.enter_context(tc.tile_pool(name="persist", bufs=1))
        xpool = top.enter_context(tc.tile_pool(name="xpool", bufs=4))
        ppool = top.enter_context(tc.tile_pool(name="ppool", bufs=6))
        rcpool = top.enter_context(tc.tile_pool(name="rcpool", bufs=2))
        otpool = top.enter_context(tc.tile_pool(name="otpool", bufs=2))
        ps_s = top.enter_context(tc.tile_pool(name="ps_s", bufs=2, space="PSUM"))
        ps_o = top.enter_context(tc.tile_pool(name="ps_o", bufs=3, space="PSUM"))
        ps_m = top.enter_context(tc.tile_pool(name="ps_m", bufs=1, space="PSUM"))

        # ---------------- persistent SBUF ----------------
        wqk_sb = persist.tile([128, NDCH, 2 * DL], BF16, name="wqk_sb")
        wv_sb = persist.tile([128, NDCH, DL], BF16, name="wv_sb")
        wp_sb = persist.tile([128, 4, D], BF16, name="wp_sb")
        qT = [persist.tile([128, T], BF16, name=f"qT{i}") for i in range(4)]
        kT = [persist.tile([128, T], BF16, name=f"kT{i}") for i in range(4)]
        # per head h: [V_h (64 cols) | ones (64 cols)] -> PV with this as
        # stationary operand accumulates O^T in PSUM rows 0:63 and the
        # broadcast softmax denominator in rows 64:127.
        vs = [persist.tile([128, HL, 128], BF16, name=f"vs{t}")
              for t in range(NKC)]
        yT = [persist.tile([128, T], BF16, name=f"yT{i}") for i in range(4)]
        bias_sb = persist.tile([128, 8], F32, name="bias_sb")
        bv_f = persist.tile([1, DL], F32, name="bv_f")
        bv_r = persist.tile([1, DL], F32R, name="bv_r")
        ones_f = persist.tile([1, 128], F32, name="ones_f")
        ones_r = persist.tile([1, 128], F32R, name="ones_r")
        warm = persist.tile([128, TQ], BF16, name="warm")
        scratch = persist.tile([128, 32], F32, name="scratch")
        maskt = persist.tile([TKC, 2, TKC], BF16, name="maskt")

        # PE warmup (keep HAM un-throttled until real matmuls arrive) and
        # early exp table load, while the first DMAs are in flight.
        nc.vector.memset(warm[:], 1.0)
        nc.scalar.activation(scratch[:, 0:16], warm[:, 0:16], EXP, scale=0.125)
        NWARM = 24
        ps_w = ps_m.tile([128, TQ], F32, tag="psm", name="warmps")
        for w in range(NWARM):
            nc.tensor.matmul(ps_w[:], warm[:, 0:128], warm[:],
                             start=(w == 0), stop=(w == NWARM - 1))
        nc.vector.tensor_copy(scratch[:, 16:32], ps_w[:, 0:16])
        nc.gpsimd.dma_start(sink_d, scratch[:])

        nc.vector.memset(ones_f[:], 1.0)
        nc.vector.tensor_copy(ones_r[:], ones_f[:])
        for t in range(NKC):
            nc.vector.memset(vs[t][:, :, 64:128], 1.0)

        # ---------------- DMAs ----------------
        nc.gpsimd.dma_start(bias_sb[:], bqk_d)
        nc.gpsimd.dma_start(bv_f[:], bv_d)
        nc.vector.tensor_copy(bv_r[:], bv_f[:])
        if causal:
            nc.gpsimd.dma_start(
                maskt[:], maskt_d.rearrange("p (two m) -> p two m", two=2))
        nc.gpsimd.dma_start(
            wqk_sb[:], wqk_d.rearrange("(dd p) m -> p dd m", p=128))
        nc.gpsimd.dma_start(
            wv_sb[:], wv_d.rearrange("(dd p) m -> p dd m", p=128))
        nc.gpsimd.dma_start(
            wp_sb[:], wp_d.rearrange("(kk p) m -> p kk m", p=128))

        xsrc = xT_d.rearrange("(dd p) t -> p dd t", p=128)
        xall = [None] * NQB

        def load_x(j):
            xall[j] = xpool.tile([128, NDCH, TQ], BF16, tag="xall",
                                 name=f"x{j}")
            nc.sync.dma_start(xall[j][:], xsrc[:, :, j * TQ:(j + 1) * TQ])

        # ---------------- step generators ----------------
        def qkv_steps(j):
            """12 closures: 8 QK m-tile groups + 4 V chunk groups."""
            jsl = slice(j * TQ, (j + 1) * TQ)
            steps = []

            def qk_group(m, j=j, jsl=jsl):
                ps = ps_m.tile([128, TQ], F32, tag="psm", name=f"qk{j}_{m}")
                for d in range(NDCH):
                    nc.tensor.matmul(
                        ps[:], wqk_sb[:, d, m * 128:(m + 1) * 128],
                        xall[j][:, d, :], start=(d == 0), stop=(d == NDCH - 1))
                dest = qT[m] if m < 4 else kT[m - 4]
                nc.vector.tensor_scalar_add(dest[:, jsl], ps[:],
                                            bias_sb[:, m:m + 1])

            def v_group(c4, j=j):
                tt = 4 * j + c4
                ps = ps_m.tile([128, DL], F32, tag="psm", name=f"v{tt}")
                for d in range(NDCH):
                    nc.tensor.matmul(
                        ps[:], xall[j][:, d, c4 * 128:(c4 + 1) * 128],
                        wv_sb[:, d, :], start=(d == 0), stop=False)
                nc.tensor.matmul(ps[:], ones_r[:, 0:128], bv_r[:],
                                 start=False, stop=True)
                nc.vector.tensor_copy(
                    vs[tt][:, :, 0:64],
                    ps[:].rearrange("p (h v) -> p h v", h=HL))

            for m in range(8):
                steps.append(lambda m=m: qk_group(m))
            for c4 in range(4):
                steps.append(lambda c4=c4: v_group(c4))
            return steps

        def proj_steps(j):
            """8 closures: per t-tile (4) x per half (2)."""
            steps = []
            ot = [None, None, None, None]

            def pstep(t4, nb, j=j):
                t = 4 * j + t4
                if nb == 0:
                    ot[t4] = otpool.tile([128, D], F32, tag="ot",
                                         name=f"ot{t}")
                ps = ps_m.tile([128, TQ], F32, tag="psm", name=f"p3_{t}_{nb}")
                for k in range(4):
                    nc.tensor.matmul(
                        ps[:], yT[k][:, t * 128:(t + 1) * 128],
                        wp_sb[:, k, nb * TQ:(nb + 1) * TQ],
                        start=(k == 0), stop=(k == 3))
                nc.vector.tensor_copy(ot[t4][:, nb * TQ:(nb + 1) * TQ], ps[:])
                if nb == 1:
                    nc.gpsimd.dma_start(out_d[t * 128:(t + 1) * 128, :],
                                        ot[t4][:])

            for t4 in range(4):
                for nb in range(2):
                    steps.append(lambda t4=t4, nb=nb: pstep(t4, nb))
            return steps

        # ---------------- fused pipeline ----------------
        load_x(0)
        for step in qkv_steps(0):     # prologue: QKV for block 0
            step()

        for j in range(NQB):
            jsl = slice(j * TQ, (j + 1) * TQ)
            fillers = deque()
            # deep x prefetch: xall[j+1] must land well before its QKV steps
            # run (the PE's hoisted LDWEIGHTS can otherwise race the DMA).
            if j == 0:
                load_x(1)
                load_x(2)
            elif j + 2 < NQB:
                load_x(j + 2)
            if j + 1 < NQB:
                fillers.extend(qkv_steps(j + 1))
            if j >= 1:
                fillers.extend(proj_steps(j - 1))

            cs = list(range(4 * (j + 1))) if causal else list(range(NKC))
            for i in range(4):        # head pair (2i, 2i+1)
                poA = ps_o.tile([128, TQ], F32, tag="po", name=f"poA{j}_{i}")
                poB = ps_o.tile([128, TQ], F32, tag="po", name=f"poB{j}_{i}")

                pend = None           # PV(c) emitted after exp(c+1)
                for ci, c in enumerate(cs):
                    csl = slice(c * TKC, (c + 1) * TKC)
                    s = c - 4 * j if (causal and c >= 4 * j) else None
                    lo = s * TKC if s else 0
                    ss = ps_s.tile([TKC, 2, TQ], F32, tag="ss",
                                   name=f"ss{j}_{i}_{c}")
                    # two row-tiled K=64 matmuls -> run concurrently
                    nc.tensor.matmul(
                        ss[:, 0, lo:TQ], kT[i][0:64, csl],
                        qT[i][0:64, j * TQ + lo:(j + 1) * TQ],
                        start=True, stop=True)
                    nc.tensor.matmul(
                        ss[:, 1, lo:TQ], kT[i][64:128, csl],
                        qT[i][64:128, j * TQ + lo:(j + 1) * TQ],
                        start=True, stop=True)
                    pt = ppool.tile([TKC, 2, TQ], BF16, tag="pt",
                                    name=f"pt{j}_{i}_{c}")
                    nc.scalar.activation(pt[:, :, lo:TQ], ss[:, :, lo:TQ],
                                         EXP, scale=0.125)
                    if s is not None:
                        nc.vector.tensor_mul(pt[:, :, lo:lo + TKC],
                                             pt[:, :, lo:lo + TKC], maskt[:])
                    if pend is not None:
                        pc, ppt, plo = pend
                        st = (ci == 1)
                        nc.tensor.matmul(
                            poA[:, plo:TQ], vs[pc][:, 2 * i, :],
                            ppt[:, 0, plo:TQ], start=st, stop=False)
                        nc.tensor.matmul(
                            poB[:, plo:TQ], vs[pc][:, 2 * i + 1, :],
                            ppt[:, 1, plo:TQ], start=st, stop=False)
                    if fillers:
                        fillers.popleft()()
                    pend = (c, pt, lo)

                pc, ppt, plo = pend
                one = (len(cs) == 1)
                nc.tensor.matmul(poA[:, plo:TQ], vs[pc][:, 2 * i, :],
                                 ppt[:, 0, plo:TQ], start=one, stop=True)
                nc.tensor.matmul(poB[:, plo:TQ], vs[pc][:, 2 * i + 1, :],
                                 ppt[:, 1, plo:TQ], start=one, stop=True)

                # normalize: rows 64:127 of po already hold the broadcast
                # denominator; copy to SBUF, reciprocal, multiply.  Every DVE
                # op keeps its operands' base partitions aligned (mismatched
                # input bases in one DVE op silently read wrong partitions).
                dsb = rcpool.tile([64, 2 * TQ], F32, tag="dsb", name=f"ds{j}_{i}")
                rc = rcpool.tile([64, 2 * TQ], F32, tag="rc", name=f"rc{j}_{i}")
                nc.vector.tensor_copy(dsb[:, 0:TQ], poA[64:128, :])
                nc.vector.tensor_copy(dsb[:, TQ:2 * TQ], poB[64:128, :])
                nc.vector.reciprocal_approx_fast(out=rc[:], in_=dsb[:])
                nc.vector.tensor_mul(yT[i][0:64, jsl], poA[0:64, :],
                                     rc[:, 0:TQ])
                nc.vector.tensor_mul(yT[i][64:128, jsl], poB[0:64, :],
                                     rc[:, TQ:2 * TQ])

            while fillers:            # flush leftover interleaved steps
                fillers.popleft()()

        for step in proj_steps(NQB - 1):   # epilogue: last block's projection
            step()

    nc.compile()
    return nc


def _get_nc(causal: bool):
    if causal not in _CACHE:
        _CACHE[causal] = _build(causal)
    return _CACHE[causal]


def _host_mask_tri() -> np.ndarray:
    i = np.arange(TKC)[:, None]
    jj = np.arange(TKC)[None, :]
    tri = (jj >= i).astype(np.float32)
    return np.ascontiguousarray(
        np.concatenate([tri, tri], axis=1).astype(ml_dtypes.bfloat16))


def _make_in_maps(x, W_qkv, b_qkv, W_proj):
    mask_np = _host_mask_tri()
    bf = ml_dtypes.bfloat16
    in_maps = []
    for core in range(N_CORES):
        b, g = core // 2, core % 2
        qc = slice(g * DL, (g + 1) * DL)
        kc = slice(D + g * DL, D + (g + 1) * DL)
        vc = slice(2 * D + g * DL, 2 * D + (g + 1) * DL)
        in_maps.append({
            "xT": np.ascontiguousarray(x[b].T.astype(bf)),
            "wqk": np.ascontiguousarray(np.concatenate(
                [W_qkv[:, qc], W_qkv[:, kc]], axis=1).astype(bf)),
            "wv": np.ascontiguousarray(W_qkv[:, vc].astype(bf)),
            "bqk": np.ascontiguousarray(
                np.concatenate([b_qkv[qc], b_qkv[kc]]).reshape(8, 128).T),
            "bv": np.ascontiguousarray(b_qkv[vc].reshape(1, DL)),
            "wproj": np.ascontiguousarray(
                W_proj[g * DL:(g + 1) * DL, :].astype(bf)),
            "maskt": mask_np,
        })
    return in_maps


def kernel(x, mask, W_qkv, b_qkv, W_proj, b_proj):
    x = np.asarray(x, dtype=np.float32)
    mask2d = np.asarray(mask, dtype=np.int32).reshape(T, T)
    W_qkv = np.asarray(W_qkv, dtype=np.float32)
    b_qkv = np.asarray(b_qkv, dtype=np.float32)
    W_proj = np.asarray(W_proj, dtype=np.float32)
    b_proj = np.asarray(b_proj, dtype=np.float32)

    if np.array_equal(mask2d, np.tril(np.ones((T, T), dtype=np.int32))):
        causal = True
    elif np.all(mask2d == 1):
        causal = False
    else:
        raise NotImplementedError("only causal (tril) or all-ones masks")

    nc = _get_nc(causal)
    in_maps = _make_in_maps(x, W_qkv, b_qkv, W_proj)
    res = run_bass_kernel_spmd(nc, in_maps, core_ids=list(range(N_CORES)))
    out = np.empty((B, T, D), dtype=np.float32)
    for b in range(B):
        out[b] = (res.results[2 * b]["out"] + res.results[2 * b + 1]["out"]
                  + b_proj[None, :])
    return out


# revision 21
# speedup vs baseline: 1.3197x; 1.2627x over previous
"""Multi-head causal self-attention for TRN2, 8 NeuronCores.

Sharding: core i handles (batch b = i//2, head-group g = i%2); each head-group
is 8 of the 16 heads.  Everything on-device is computed in "transposed" space
(features on partitions, positions on the free dim) so no transposes are
needed.  Single fused pipeline per query block j (no separate phases):

  QKV(j+1) and proj(j-1) matmuls are interleaved into attention(j) so the PE
  never idles while the scalar engine chews through the softmax exps.

  scores:  S^T(h) = kT.T @ qT per head, K=64.  The two heads of a pair are
           packed into partitions 0:64 / 64:128 of one kT/qT tile and issued
           as two row-tiled matmuls (tile_position (0,0)/(64,0)) that run
           CONCURRENTLY in the PE array -> ~2x score throughput.
  softmax: one exp per (chunk, pair) over [128, <=1024] PSUM; on diagonal
           chunks the column range is restricted to the causally live region
           and only the 128-wide triangle gets a mask multiply.
  PV:      stationary operand is [V_h (64 cols) | ones (64 cols)] via a
           strided AP, so PSUM rows 0:63 accumulate O^T and rows 64:127 all
           accumulate the softmax denominator -- already broadcast.  The
           normalize is then just reciprocal_approx_fast + tensor_tensor
           multiply on DVE (no PE broadcast matmul, no single-lane ops).
  proj:    y^T.T @ W_proj in bf16, partial outputs summed on host.

All inputs staged as bf16 on the host (halves DMA, removes all on-device
weight/x casts).  b_qkv handled via DVE per-partition bias add (Q/K) and a
K=1 ones matmul (V).  b_proj added on host.
"""

import numpy as np
import ml_dtypes
from collections import deque
from contextlib import ExitStack

import concourse.bass as bass
import concourse.mybir as mybir
import concourse.tile as tile
from concourse import bacc
from concourse.bass_utils import run_bass_kernel_spmd

B, T, D, H = 4, 2048, 1024, 16
DK = 64            # head dim
HL = 8             # heads per core
DL = HL * DK       # 512 local head dims per core
N_CORES = 8

F32 = mybir.dt.float32
F32R = mybir.dt.float32r
BF16 = mybir.dt.bfloat16
EXP = mybir.ActivationFunctionType.Exp

TQ = 512           # query block size
TKC = 128          # key chunk size
NQB = T // TQ      # 4
NKC = T // TKC     # 16
NDCH = D // 128    # 8 contraction chunks over D
_CACHE = {}


def _build(causal: bool, vbias_zero: bool = True):
    nc = bacc.Bacc("TRN2", target_bir_lowering=False, debug=False,
                   num_devices=N_CORES)
    xT_d = nc.dram_tensor("xT", [D, T], BF16, kind="ExternalInput").ap()
    wqk_d = nc.dram_tensor("wqk", [D, 2 * DL], BF16, kind="ExternalInput").ap()
    wv_d = nc.dram_tensor("wv", [D, DL], BF16, kind="ExternalInput").ap()
    bqk_d = nc.dram_tensor("bqk", [128, 8], F32, kind="ExternalInput").ap()
    bv_d = nc.dram_tensor("bv", [1, DL], F32, kind="ExternalInput").ap()
    wp_d = nc.dram_tensor("wproj", [DL, D], BF16, kind="ExternalInput").ap()
    maskt_d = nc.dram_tensor("maskt", [TKC, 2 * TKC], BF16,
                             kind="ExternalInput").ap()
    out_d = nc.dram_tensor("out", [T, D], F32, kind="ExternalOutput").ap()
    sink_d = nc.dram_tensor("sink", [128, 32], F32, kind="ExternalOutput").ap()

    with tile.TileContext(nc) as tc, ExitStack() as top:
        persist = top.enter_context(tc.tile_pool(name="persist", bufs=1))
        xpool = top.enter_context(tc.tile_pool(name="xpool", bufs=4))
        ppool = top.enter_context(tc.tile_pool(name="ppool", bufs=6))
        rcpool = top.enter_context(tc.tile_pool(name="rcpool", bufs=2))
        otpool = top.enter_context(tc.tile_pool(name="otpool", bufs=2))
        ps_s = top.enter_context(tc.tile_pool(name="ps_s", bufs=2, space="PSUM"))
        ps_o = top.enter_context(tc.tile_pool(name="ps_o", bufs=3, space="PSUM"))
        ps_m = top.enter_context(tc.tile_pool(name="ps_m", bufs=1, space="PSUM"))

        # ---------------- persistent SBUF ----------------
        wqk_sb = persist.tile([128, NDCH, 2 * DL], BF16, name="wqk_sb")
        wv_sb = persist.tile([128, NDCH, DL], BF16, name="wv_sb")
        wp_sb = persist.tile([128, 4, D], BF16, name="wp_sb")
        qT = [persist.tile([128, T], BF16, name=f"qT{i}") for i in range(4)]
        kT = [persist.tile([128, T], BF16, name=f"kT{i}") for i in range(4)]
        # per head h: [V_h (64 cols) | ones (64 cols)] -> PV with this as
        # stationary operand accumulates O^T in PSUM rows 0:63 and the
        # broadcast softmax denominator in rows 64:127.
        vs = [persist.tile([128, HL, 128], BF16, name=f"vs{t}")
              for t in range(NKC)]
        yT = [persist.tile([128, T], BF16, name=f"yT{i}") for i in range(4)]
        bias_sb = persist.tile([128, 8], F32, name="bias_sb")
        if not vbias_zero:
            bv_f = persist.tile([1, DL], F32, name="bv_f")
            bv_r = persist.tile([1, DL], BF16, name="bv_r")
            ones_r = persist.tile([1, 128], BF16, name="ones_r")
        warm = persist.tile([128, TQ], BF16, name="warm")
        scratch = persist.tile([128, 32], F32, name="scratch")
        maskt = persist.tile([TKC, 2, TKC], BF16, name="maskt")

        # PE warmup (keep HAM un-throttled until real matmuls arrive) and
        # early exp table load, while the first DMAs are in flight.
        nc.vector.memset(warm[:], 1.0)
        nc.scalar.activation(scratch[:, 0:16], warm[:, 0:16], EXP, scale=0.125)
        NWARM = 48
        ps_w = ps_m.tile([128, TQ], F32, tag="psm", name="warmps")
        for w in range(NWARM):
            nc.tensor.matmul(ps_w[:], warm[:, 0:128], warm[:],
                             start=(w == 0), stop=(w == NWARM - 1))
        nc.vector.tensor_copy(scratch[:, 16:32], ps_w[:, 0:16])
        nc.gpsimd.dma_start(sink_d, scratch[:])

        if not vbias_zero:
            nc.vector.memset(ones_r[:], 1.0)
        for t in range(NKC):
            nc.vector.memset(vs[t][:, :, 64:128], 1.0)

        # ---------------- DMAs ----------------
        nc.gpsimd.dma_start(bias_sb[:], bqk_d)
        if not vbias_zero:
            nc.gpsimd.dma_start(bv_f[:], bv_d)
            nc.vector.tensor_copy(bv_r[:], bv_f[:])
        if causal:
            nc.gpsimd.dma_start(
                maskt[:], maskt_d.rearrange("p (two m) -> p two m", two=2))
        nc.gpsimd.dma_start(
            wqk_sb[:], wqk_d.rearrange("(dd p) m -> p dd m", p=128))
        nc.gpsimd.dma_start(
            wv_sb[:], wv_d.rearrange("(dd p) m -> p dd m", p=128))
        nc.gpsimd.dma_start(
            wp_sb[:], wp_d.rearrange("(kk p) m -> p kk m", p=128))

        xsrc = xT_d.rearrange("(dd p) t -> p dd t", p=128)
        xall = [None] * NQB

        def load_x(j):
            xall[j] = xpool.tile([128, NDCH, TQ], BF16, tag="xall",
                                 name=f"x{j}")
            nc.sync.dma_start(xall[j][:], xsrc[:, :, j * TQ:(j + 1) * TQ])

        # ---------------- step generators ----------------
        def qkv_steps(j):
            """12 closures: 8 QK m-tile groups + 4 V chunk groups."""
            jsl = slice(j * TQ, (j + 1) * TQ)
            steps = []

            def qk_group(m, j=j, jsl=jsl):
                ps = ps_m.tile([128, TQ], F32, tag="psm", name=f"qk{j}_{m}")
                for d in range(NDCH):
                    nc.tensor.matmul(
                        ps[:], wqk_sb[:, d, m * 128:(m + 1) * 128],
                        xall[j][:, d, :], start=(d == 0), stop=(d == NDCH - 1))
                dest = qT[m] if m < 4 else kT[m - 4]
                nc.vector.tensor_scalar_add(dest[:, jsl], ps[:],
                                            bias_sb[:, m:m + 1])

            def v_group(c4, j=j):
                tt = 4 * j + c4
                ps = ps_m.tile([128, DL], F32, tag="psm", name=f"v{tt}")
                for d in range(NDCH):
                    nc.tensor.matmul(
                        ps[:], xall[j][:, d, c4 * 128:(c4 + 1) * 128],
                        wv_sb[:, d, :], start=(d == 0),
                        stop=(vbias_zero and d == NDCH - 1))
                if not vbias_zero:
                    nc.tensor.matmul(ps[:], ones_r[:, 0:128], bv_r[:],
                                     start=False, stop=True)
                nc.vector.tensor_copy(
                    vs[tt][:, :, 0:64],
                    ps[:].rearrange("p (h v) -> p h v", h=HL))

            for m in range(8):
                steps.append(lambda m=m: qk_group(m))
            for c4 in range(4):
                steps.append(lambda c4=c4: v_group(c4))
            return steps

        def proj_steps(j, pool=None):
            """8 closures: per t-tile (4) x per half (2)."""
            steps = []
            ot = [None, None, None, None]

            def pstep(t4, nb, j=j):
                t = 4 * j + t4
                if nb == 0:
                    ot[t4] = otpool.tile([128, D], F32, tag="ot",
                                         name=f"ot{t}")
                if pool is None:
                    ps = ps_m.tile([128, TQ], F32, tag="psm",
                                   name=f"p3_{t}_{nb}")
                else:
                    ps = pool.tile([128, TQ], F32, tag="po",
                                   name=f"p3_{t}_{nb}")
                for k in range(4):
                    nc.tensor.matmul(
                        ps[:], yT[k][:, t * 128:(t + 1) * 128],
                        wp_sb[:, k, nb * TQ:(nb + 1) * TQ],
                        start=(k == 0), stop=(k == 3))
                nc.vector.tensor_copy(ot[t4][:, nb * TQ:(nb + 1) * TQ], ps[:])
                if nb == 1:
                    nc.gpsimd.dma_start(out_d[t * 128:(t + 1) * 128, :],
                                        ot[t4][:])

            for t4 in range(4):
                for nb in range(2):
                    steps.append(lambda t4=t4, nb=nb: pstep(t4, nb))
            return steps

        # ---------------- fused pipeline ----------------
        load_x(0)
        pro = qkv_steps(0)            # prologue: QKV for block 0; emit the
        order = [0, 4, 8, 9, 10, 11, 1, 5, 2, 6, 3, 7]   # pair-0 deps first
        for ix in order:
            pro[ix]()

        for j in range(NQB):
            jsl = slice(j * TQ, (j + 1) * TQ)
            fillers = deque()
            # deep x prefetch: xall[j+1] must land well before its QKV steps
            # run (the PE's hoisted LDWEIGHTS can otherwise race the DMA).
            if j == 0:
                load_x(1)
                load_x(2)
            elif j + 2 < NQB:
                load_x(j + 2)
            if j + 1 < NQB:
                fillers.extend(qkv_steps(j + 1))
            if j >= 1:
                fillers.extend(proj_steps(j - 1))

            cs = list(range(4 * (j + 1))) if causal else list(range(NKC))
            for i in range(4):        # head pair (2i, 2i+1)
                poA = ps_o.tile([128, TQ], F32, tag="po", name=f"poA{j}_{i}")
                poB = ps_o.tile([128, TQ], F32, tag="po", name=f"poB{j}_{i}")

                pend = None           # PV(c) emitted after exp(c+1)
                for ci, c in enumerate(cs):
                    csl = slice(c * TKC, (c + 1) * TKC)
                    s = c - 4 * j if (causal and c >= 4 * j) else None
                    lo = s * TKC if s else 0
                    ss = ps_s.tile([TKC, 2, TQ], F32, tag="ss",
                                   name=f"ss{j}_{i}_{c}")
                    # two row-tiled K=64 matmuls -> run concurrently
                    nc.tensor.matmul(
                        ss[:, 0, lo:TQ], kT[i][0:64, csl],
                        qT[i][0:64, j * TQ + lo:(j + 1) * TQ],
                        start=True, stop=True)
                    nc.tensor.matmul(
                        ss[:, 1, lo:TQ], kT[i][64:128, csl],
                        qT[i][64:128, j * TQ + lo:(j + 1) * TQ],
                        start=True, stop=True)
                    pt = ppool.tile([TKC, 2, TQ], BF16, tag="pt",
                                    name=f"pt{j}_{i}_{c}")
                    nc.scalar.activation(pt[:, :, lo:TQ], ss[:, :, lo:TQ],
                                         EXP, scale=0.125)
                    if s is not None:
                        nc.vector.tensor_mul(pt[:, :, lo:lo + TKC],
                                             pt[:, :, lo:lo + TKC], maskt[:])
                    if pend is not None:
                        pc, ppt, plo = pend
                        st = (ci == 1)
                        nc.tensor.matmul(
                            poA[:, plo:TQ], vs[pc][:, 2 * i, :],
                            ppt[:, 0, plo:TQ], start=st, stop=False)
                        nc.tensor.matmul(
                            poB[:, plo:TQ], vs[pc][:, 2 * i + 1, :],
                            ppt[:, 1, plo:TQ], start=st, stop=False)
                    if fillers:
                        fillers.popleft()()
                    pend = (c, pt, lo)

                pc, ppt, plo = pend
                one = (len(cs) == 1)
                nc.tensor.matmul(poA[:, plo:TQ], vs[pc][:, 2 * i, :],
                                 ppt[:, 0, plo:TQ], start=one, stop=True)
                nc.tensor.matmul(poB[:, plo:TQ], vs[pc][:, 2 * i + 1, :],
                                 ppt[:, 1, plo:TQ], start=one, stop=True)

                # normalize: rows 64:127 of po already hold the broadcast
                # denominator; copy to SBUF, reciprocal, multiply.  Every DVE
                # op keeps its operands' base partitions aligned (mismatched
                # input bases in one DVE op silently read wrong partitions).
                dsb = rcpool.tile([64, 2 * TQ], F32, tag="dsb", name=f"ds{j}_{i}")
                rc = rcpool.tile([64, 2 * TQ], F32, tag="rc", name=f"rc{j}_{i}")
                nc.vector.tensor_copy(dsb[:, 0:TQ], poA[64:128, :])
                nc.vector.tensor_copy(dsb[:, TQ:2 * TQ], poB[64:128, :])
                nc.vector.reciprocal_approx_fast(out=rc[:], in_=dsb[:])
                nc.vector.tensor_mul(yT[i][0:64, jsl], poA[0:64, :],
                                     rc[:, 0:TQ])
                nc.vector.tensor_mul(yT[i][64:128, jsl], poB[0:64, :],
                                     rc[:, TQ:2 * TQ])

            while fillers:            # flush leftover interleaved steps
                fillers.popleft()()

        for step in proj_steps(NQB - 1, pool=ps_o):   # epilogue
            step()

    nc.compile()
    return nc


def _get_nc(causal: bool, vbias_zero: bool = True):
    key = (causal, vbias_zero)
    if key not in _CACHE:
        _CACHE[key] = _build(causal, vbias_zero)
    return _CACHE[key]


def _host_mask_tri() -> np.ndarray:
    i = np.arange(TKC)[:, None]
    jj = np.arange(TKC)[None, :]
    tri = (jj >= i).astype(np.float32)
    return np.ascontiguousarray(
        np.concatenate([tri, tri], axis=1).astype(ml_dtypes.bfloat16))


def _make_in_maps(x, W_qkv, b_qkv, W_proj):
    mask_np = _host_mask_tri()
    bf = ml_dtypes.bfloat16
    in_maps = []
    for core in range(N_CORES):
        b, g = core // 2, core % 2
        qc = slice(g * DL, (g + 1) * DL)
        kc = slice(D + g * DL, D + (g + 1) * DL)
        vc = slice(2 * D + g * DL, 2 * D + (g + 1) * DL)
        in_maps.append({
            "xT": np.ascontiguousarray(x[b].T.astype(bf)),
            "wqk": np.ascontiguousarray(np.concatenate(
                [W_qkv[:, qc], W_qkv[:, kc]], axis=1).astype(bf)),
            "wv": np.ascontiguousarray(W_qkv[:, vc].astype(bf)),
            "bqk": np.ascontiguousarray(
                np.concatenate([b_qkv[qc], b_qkv[kc]]).reshape(8, 128).T),
            "bv": np.ascontiguousarray(b_qkv[vc].reshape(1, DL)),
            "wproj": np.ascontiguousarray(
                W_proj[g * DL:(g + 1) * DL, :].astype(bf)),
            "maskt": mask_np,
        })
    return in_maps


def kernel(x, mask, W_qkv, b_qkv, W_proj, b_proj):
    x = np.asarray(x, dtype=np.float32)
    mask2d = np.asarray(mask, dtype=np.int32).reshape(T, T)
    W_qkv = np.asarray(W_qkv, dtype=np.float32)
    b_qkv = np.asarray(b_qkv, dtype=np.float32)
    W_proj = np.asarray(W_proj, dtype=np.float32)
    b_proj = np.asarray(b_proj, dtype=np.float32)

    if np.array_equal(mask2d, np.tril(np.ones((T, T), dtype=np.int32))):
        causal = True
    elif np.all(mask2d == 1):
        causal = False
    else:
        raise NotImplementedError("only causal (tril) or all-ones masks")

    vz = not np.any(b_qkv[2 * D:])
    nc = _get_nc(causal, vz)
    in_maps = _make_in_maps(x, W_qkv, b_qkv, W_proj)
    res = run_bass_kernel_spmd(nc, in_maps, core_ids=list(range(N_CORES)))
    out = np.empty((B, T, D), dtype=np.float32)
    for b in range(B):
        out[b] = (res.results[2 * b]["out"] + res.results[2 * b + 1]["out"]
                  + b_proj[None, :])
    return out


# revision 22
# speedup vs baseline: 1.3350x; 1.0116x over previous
"""Multi-head causal self-attention for TRN2, 8 NeuronCores.

Sharding: core i handles (batch b = i//2, head-group g = i%2); each head-group
is 8 of the 16 heads.  Everything on-device is computed in "transposed" space
(features on partitions, positions on the free dim) so no transposes are
needed.  Single fused pipeline per query block j (no separate phases):

  QKV(j+1) and proj(j-1) matmuls are interleaved into attention(j) so the PE
  never idles while the scalar engine chews through the softmax exps.

  scores:  S^T(h) = kT.T @ qT per head, K=64.  The two heads of a pair are
           packed into partitions 0:64 / 64:128 of one kT/qT tile and issued
           as two row-tiled matmuls (tile_position (0,0)/(64,0)) that run
           CONCURRENTLY in the PE array -> ~2x score throughput.
  softmax: one exp per (chunk, pair) over [128, <=1024] PSUM; on diagonal
           chunks the column range is restricted to the causally live region
           and only the 128-wide triangle gets a mask multiply.
  PV:      stationary operand is [V_h (64 cols) | ones (64 cols)] via a
           strided AP, so PSUM rows 0:63 accumulate O^T and rows 64:127 all
           accumulate the softmax denominator -- already broadcast.  The
           normalize is then just reciprocal_approx_fast + tensor_tensor
           multiply on DVE (no PE broadcast matmul, no single-lane ops).
  proj:    y^T.T @ W_proj in bf16, partial outputs summed on host.

All inputs staged as bf16 on the host (halves DMA, removes all on-device
weight/x casts).  b_qkv handled via DVE per-partition bias add (Q/K) and a
K=1 ones matmul (V).  b_proj added on host.
"""

import numpy as np
import ml_dtypes
from collections import deque
from contextlib import ExitStack

import concourse.bass as bass
import concourse.mybir as mybir
import concourse.tile as tile
from concourse import bacc
from concourse.bass_utils import run_bass_kernel_spmd

B, T, D, H = 4, 2048, 1024, 16
DK = 64            # head dim
HL = 8             # heads per core
DL = HL * DK       # 512 local head dims per core
N_CORES = 8

F32 = mybir.dt.float32
F32R = mybir.dt.float32r
BF16 = mybir.dt.bfloat16
EXP = mybir.ActivationFunctionType.Exp

TQ = 512           # query block size
TKC = 128          # key chunk size
NQB = T // TQ      # 4
NKC = T // TKC     # 16
NDCH = D // 128    # 8 contraction chunks over D
_CACHE = {}


def _build(causal: bool, vbias_zero: bool = True):
    nc = bacc.Bacc("TRN2", target_bir_lowering=False, debug=False,
                   num_devices=N_CORES)
    xT_d = nc.dram_tensor("xT", [D, T], BF16, kind="ExternalInput").ap()
    wqk_d = nc.dram_tensor("wqk", [D, 2 * DL], BF16, kind="ExternalInput").ap()
    wv_d = nc.dram_tensor("wv", [D, DL], BF16, kind="ExternalInput").ap()
    bqk_d = nc.dram_tensor("bqk", [128, 8], F32, kind="ExternalInput").ap()
    bv_d = nc.dram_tensor("bv", [1, DL], F32, kind="ExternalInput").ap()
    wp_d = nc.dram_tensor("wproj", [DL, D], BF16, kind="ExternalInput").ap()
    maskt_d = nc.dram_tensor("maskt", [TKC, 2 * TKC], BF16,
                             kind="ExternalInput").ap()
    out_d = nc.dram_tensor("out", [T, D], F32, kind="ExternalOutput").ap()
    sink_d = nc.dram_tensor("sink", [128, 32], F32, kind="ExternalOutput").ap()

    with tile.TileContext(nc) as tc, ExitStack() as top:
        persist = top.enter_context(tc.tile_pool(name="persist", bufs=1))
        xpool = top.enter_context(tc.tile_pool(name="xpool", bufs=4))
        ppool = top.enter_context(tc.tile_pool(name="ppool", bufs=6))
        rcpool = top.enter_context(tc.tile_pool(name="rcpool", bufs=2))
        otpool = top.enter_context(tc.tile_pool(name="otpool", bufs=2))
        ps_s = top.enter_context(tc.tile_pool(name="ps_s", bufs=2, space="PSUM"))
        ps_o = top.enter_context(tc.tile_pool(name="ps_o", bufs=3, space="PSUM"))
        ps_m = top.enter_context(tc.tile_pool(name="ps_m", bufs=1, space="PSUM"))

        # ---------------- persistent SBUF ----------------
        wqk_sb = persist.tile([128, NDCH, 2 * DL], BF16, name="wqk_sb")
        wv_sb = persist.tile([128, NDCH, DL], BF16, name="wv_sb")
        wp_sb = persist.tile([128, 4, D], BF16, name="wp_sb")
        qT = [persist.tile([128, T], BF16, name=f"qT{i}") for i in range(4)]
        kT = [persist.tile([128, T], BF16, name=f"kT{i}") for i in range(4)]
        # per head h: [V_h (64 cols) | ones (64 cols)] -> PV with this as
        # stationary operand accumulates O^T in PSUM rows 0:63 and the
        # broadcast softmax denominator in rows 64:127.
        vs = [persist.tile([128, HL, 128], BF16, name=f"vs{t}")
              for t in range(NKC)]
        yT = [persist.tile([128, T], BF16, name=f"yT{i}") for i in range(4)]
        bias_sb = persist.tile([128, 8], F32, name="bias_sb")
        if not vbias_zero:
            bv_f = persist.tile([1, DL], F32, name="bv_f")
            bv_r = persist.tile([1, DL], BF16, name="bv_r")
            ones_r = persist.tile([1, 128], BF16, name="ones_r")
        warm = persist.tile([128, TQ], BF16, name="warm")
        scratch = persist.tile([128, 32], F32, name="scratch")
        maskt = persist.tile([TKC, 2, TKC], BF16, name="maskt")

        # PE warmup (keep HAM un-throttled until real matmuls arrive) and
        # early exp table load, while the first DMAs are in flight.
        nc.vector.memset(warm[:], 1.0)
        nc.scalar.activation(scratch[:, 0:16], warm[:, 0:16], EXP, scale=0.125)
        NWARM = 48
        ps_w = ps_m.tile([128, TQ], F32, tag="psm", name="warmps")
        for w in range(NWARM):
            nc.tensor.matmul(ps_w[:], warm[:, 0:128], warm[:],
                             start=(w == 0), stop=(w == NWARM - 1))
        nc.vector.tensor_copy(scratch[:, 16:32], ps_w[:, 0:16])
        nc.gpsimd.dma_start(sink_d, scratch[:])

        if not vbias_zero:
            nc.vector.memset(ones_r[:], 1.0)
        for t in range(NKC):
            nc.vector.memset(vs[t][:, :, 64:128], 1.0)

        # ---------------- DMAs ----------------
        # weight/x loads split across idle queues so QKV(0) can start early
        wqk_src = wqk_d.rearrange("(dd p) m -> p dd m", p=128)
        nc.gpsimd.dma_start(wqk_sb[:, 0:4], wqk_src[:, 0:4])
        nc.sync.dma_start(wqk_sb[:, 4:8], wqk_src[:, 4:8])
        xsrc = xT_d.rearrange("(dd p) t -> p dd t", p=128)
        xall = [None] * NQB

        def load_x(j, split=False):
            xall[j] = xpool.tile([128, NDCH, TQ], BF16, tag="xall",
                                 name=f"x{j}")
            if split:
                nc.sync.dma_start(xall[j][:, 0:4], xsrc[:, 0:4, j * TQ:(j + 1) * TQ])
                nc.scalar.dma_start(xall[j][:, 4:8], xsrc[:, 4:8, j * TQ:(j + 1) * TQ])
            else:
                nc.sync.dma_start(xall[j][:], xsrc[:, :, j * TQ:(j + 1) * TQ])

        nc.gpsimd.dma_start(
            wv_sb[:], wv_d.rearrange("(dd p) m -> p dd m", p=128))
        nc.gpsimd.dma_start(bias_sb[:], bqk_d)
        if not vbias_zero:
            nc.gpsimd.dma_start(bv_f[:], bv_d)
            nc.vector.tensor_copy(bv_r[:], bv_f[:])
        if causal:
            nc.gpsimd.dma_start(
                maskt[:], maskt_d.rearrange("p (two m) -> p two m", two=2))
        nc.gpsimd.dma_start(
            wp_sb[:], wp_d.rearrange("(kk p) m -> p kk m", p=128))

        # ---------------- step generators ----------------
        def qkv_steps(j):
            """12 closures: 8 QK m-tile groups + 4 V chunk groups."""
            jsl = slice(j * TQ, (j + 1) * TQ)
            steps = []

            def qk_group(m, j=j, jsl=jsl):
                ps = ps_m.tile([128, TQ], F32, tag="psm", name=f"qk{j}_{m}")
                for d in range(NDCH):
                    nc.tensor.matmul(
                        ps[:], wqk_sb[:, d, m * 128:(m + 1) * 128],
                        xall[j][:, d, :], start=(d == 0), stop=(d == NDCH - 1))
                dest = qT[m] if m < 4 else kT[m - 4]
                nc.vector.tensor_scalar_add(dest[:, jsl], ps[:],
                                            bias_sb[:, m:m + 1])

            def v_group(c4, j=j):
                tt = 4 * j + c4
                ps = ps_m.tile([128, DL], F32, tag="psm", name=f"v{tt}")
                for d in range(NDCH):
                    nc.tensor.matmul(
                        ps[:], xall[j][:, d, c4 * 128:(c4 + 1) * 128],
                        wv_sb[:, d, :], start=(d == 0),
                        stop=(vbias_zero and d == NDCH - 1))
                if not vbias_zero:
                    nc.tensor.matmul(ps[:], ones_r[:, 0:128], bv_r[:],
                                     start=False, stop=True)
                nc.vector.tensor_copy(
                    vs[tt][:, :, 0:64],
                    ps[:].rearrange("p (h v) -> p h v", h=HL))

            for m in range(8):
                steps.append(lambda m=m: qk_group(m))
            for c4 in range(4):
                steps.append(lambda c4=c4: v_group(c4))
            return steps

        def proj_steps(j, pool=None):
            """8 closures: per t-tile (4) x per half (2)."""
            steps = []
            ot = [None, None, None, None]

            def pstep(t4, nb, j=j):
                t = 4 * j + t4
                if nb == 0:
                    ot[t4] = otpool.tile([128, D], F32, tag="ot",
                                         name=f"ot{t}")
                if pool is None:
                    ps = ps_m.tile([128, TQ], F32, tag="psm",
                                   name=f"p3_{t}_{nb}")
                else:
                    ps = pool.tile([128, TQ], F32, tag="po",
                                   name=f"p3_{t}_{nb}")
                for k in range(4):
                    nc.tensor.matmul(
                        ps[:], yT[k][:, t * 128:(t + 1) * 128],
                        wp_sb[:, k, nb * TQ:(nb + 1) * TQ],
                        start=(k == 0), stop=(k == 3))
                nc.vector.tensor_copy(ot[t4][:, nb * TQ:(nb + 1) * TQ], ps[:])
                if nb == 1:
                    eng = nc.gpsimd if t % 2 == 0 else nc.sync
                    eng.dma_start(out_d[t * 128:(t + 1) * 128, :], ot[t4][:])

            for t4 in range(4):
                for nb in range(2):
                    steps.append(lambda t4=t4, nb=nb: pstep(t4, nb))
            return steps

        # ---------------- fused pipeline ----------------
        load_x(0, split=True)
        pro = qkv_steps(0)            # prologue: QKV for block 0; emit the
        order = [0, 4, 8, 9, 10, 11, 1, 5, 2, 6, 3, 7]   # pair-0 deps first
        for ix in order:
            pro[ix]()

        for j in range(NQB):
            jsl = slice(j * TQ, (j + 1) * TQ)
            fillers = deque()
            # deep x prefetch: xall[j+1] must land well before its QKV steps
            # run (the PE's hoisted LDWEIGHTS can otherwise race the DMA).
            if j == 0:
                load_x(1)
                load_x(2)
            elif j + 2 < NQB:
                load_x(j + 2)
            if j + 1 < NQB:
                fillers.extend(qkv_steps(j + 1))
            if j >= 1:
                fillers.extend(proj_steps(j - 1))

            cs = list(range(4 * (j + 1))) if causal else list(range(NKC))
            for i in range(4):        # head pair (2i, 2i+1)
                poA = ps_o.tile([128, TQ], F32, tag="po", name=f"poA{j}_{i}")
                poB = ps_o.tile([128, TQ], F32, tag="po", name=f"poB{j}_{i}")

                pend = None           # PV(c) emitted after exp(c+1)
                for ci, c in enumerate(cs):
                    csl = slice(c * TKC, (c + 1) * TKC)
                    s = c - 4 * j if (causal and c >= 4 * j) else None
                    lo = s * TKC if s else 0
                    ss = ps_s.tile([TKC, 2, TQ], F32, tag="ss",
                                   name=f"ss{j}_{i}_{c}")
                    # two row-tiled K=64 matmuls -> run concurrently
                    nc.tensor.matmul(
                        ss[:, 0, lo:TQ], kT[i][0:64, csl],
                        qT[i][0:64, j * TQ + lo:(j + 1) * TQ],
                        start=True, stop=True)
                    nc.tensor.matmul(
                        ss[:, 1, lo:TQ], kT[i][64:128, csl],
                        qT[i][64:128, j * TQ + lo:(j + 1) * TQ],
                        start=True, stop=True)
                    pt = ppool.tile([TKC, 2, TQ], BF16, tag="pt",
                                    name=f"pt{j}_{i}_{c}")
                    nc.scalar.activation(pt[:, :, lo:TQ], ss[:, :, lo:TQ],
                                         EXP, scale=0.125)
                    if s is not None:
                        nc.vector.tensor_mul(pt[:, :, lo:lo + TKC],
                                             pt[:, :, lo:lo + TKC], maskt[:])
                    if pend is not None:
                        pc, ppt, plo = pend
                        st = (ci == 1)
                        nc.tensor.matmul(
                            poA[:, plo:TQ], vs[pc][:, 2 * i, :],
                            ppt[:, 0, plo:TQ], start=st, stop=False)
                        nc.tensor.matmul(
                            poB[:, plo:TQ], vs[pc][:, 2 * i + 1, :],
                            ppt[:, 1, plo:TQ], start=st, stop=False)
                    if fillers:
                        fillers.popleft()()
                    pend = (c, pt, lo)

                pc, ppt, plo = pend
                one = (len(cs) == 1)
                nc.tensor.matmul(poA[:, plo:TQ], vs[pc][:, 2 * i, :],
                                 ppt[:, 0, plo:TQ], start=one, stop=True)
                nc.tensor.matmul(poB[:, plo:TQ], vs[pc][:, 2 * i + 1, :],
                                 ppt[:, 1, plo:TQ], start=one, stop=True)

                # normalize: rows 64:127 of po already hold the broadcast
                # denominator; copy to SBUF, reciprocal, multiply.  Every DVE
                # op keeps its operands' base partitions aligned (mismatched
                # input bases in one DVE op silently read wrong partitions).
                dsb = rcpool.tile([64, 2 * TQ], F32, tag="dsb", name=f"ds{j}_{i}")
                rc = rcpool.tile([64, 2 * TQ], F32, tag="rc", name=f"rc{j}_{i}")
                nc.vector.tensor_copy(dsb[:, 0:TQ], poA[64:128, :])
                nc.vector.tensor_copy(dsb[:, TQ:2 * TQ], poB[64:128, :])
                nc.vector.reciprocal_approx_fast(out=rc[:], in_=dsb[:])
                nc.vector.tensor_mul(yT[i][0:64, jsl], poA[0:64, :],
                                     rc[:, 0:TQ])
                nc.vector.tensor_mul(yT[i][64:128, jsl], poB[0:64, :],
                                     rc[:, TQ:2 * TQ])

            while fillers:            # flush leftover interleaved steps
                fillers.popleft()()

        for step in proj_steps(NQB - 1, pool=ps_o):   # epilogue
            step()

    nc.compile()
    return nc


def _get_nc(causal: bool, vbias_zero: bool = True):
    key = (causal, vbias_zero)
    if key not in _CACHE:
        _CACHE[key] = _build(causal, vbias_zero)
    return _CACHE[key]


def _host_mask_tri() -> np.ndarray:
    i = np.arange(TKC)[:, None]
    jj = np.arange(TKC)[None, :]
    tri = (jj >= i).astype(np.float32)
    return np.ascontiguousarray(
        np.concatenate([tri, tri], axis=1).astype(ml_dtypes.bfloat16))


def _make_in_maps(x, W_qkv, b_qkv, W_proj):
    mask_np = _host_mask_tri()
    bf = ml_dtypes.bfloat16
    in_maps = []
    for core in range(N_CORES):
        b, g = core // 2, core % 2
        qc = slice(g * DL, (g + 1) * DL)
        kc = slice(D + g * DL, D + (g + 1) * DL)
        vc = slice(2 * D + g * DL, 2 * D + (g + 1) * DL)
        in_maps.append({
            "xT": np.ascontiguousarray(x[b].T.astype(bf)),
            "wqk": np.ascontiguousarray(np.concatenate(
                [W_qkv[:, qc], W_qkv[:, kc]], axis=1).astype(bf)),
            "wv": np.ascontiguousarray(W_qkv[:, vc].astype(bf)),
            "bqk": np.ascontiguousarray(
                np.concatenate([b_qkv[qc], b_qkv[kc]]).reshape(8, 128).T),
            "bv": np.ascontiguousarray(b_qkv[vc].reshape(1, DL)),
            "wproj": np.ascontiguousarray(
                W_proj[g * DL:(g + 1) * DL, :].astype(bf)),
            "maskt": mask_np,
        })
    return in_maps


def kernel(x, mask, W_qkv, b_qkv, W_proj, b_proj):
    x = np.asarray(x, dtype=np.float32)
    mask2d = np.asarray(mask, dtype=np.int32).reshape(T, T)
    W_qkv = np.asarray(W_qkv, dtype=np.float32)
    b_qkv = np.asarray(b_qkv, dtype=np.float32)
    W_proj = np.asarray(W_proj, dtype=np.float32)
    b_proj = np.asarray(b_proj, dtype=np.float32)

    if np.array_equal(mask2d, np.tril(np.ones((T, T), dtype=np.int32))):
        causal = True
    elif np.all(mask2d == 1):
        causal = False
    else:
        raise NotImplementedError("only causal (tril) or all-ones masks")

    vz = not np.any(b_qkv[2 * D:])
    nc = _get_nc(causal, vz)
    in_maps = _make_in_maps(x, W_qkv, b_qkv, W_proj)
    res = run_bass_kernel_spmd(nc, in_maps, core_ids=list(range(N_CORES)))
    out = np.empty((B, T, D), dtype=np.float32)
    for b in range(B):
        out[b] = (res.results[2 * b]["out"] + res.results[2 * b + 1]["out"]
                  + b_proj[None, :])
    return out
